# revision 9
# baseline (speedup 1.0000x reference)
"""Two-layer Keras-style GRU (reset_after=True, sigmoid/relu) + dense head
on 8 Trainium2 NeuronCores, data-parallel over batch (16 rows/core).

Transposed formulation: all per-step tensors live as [128 hidden-part,
4 ktile, 16 batch] so the recurrence matmuls are rec.T[g] = sum_k
U(k,g).T @ hT_k (stationary = U tile, moving = hT, N=16, bf16) and the
gate elementwise runs on 64-elem free dims instead of 512.  gx (input
projections) are computed chunk-wise as gxT = W.T @ xT / W2.T @ h1T and
injected into PSUM via identity matmuls, one accumulation epoch per
bank.  Layer-2 scan trails layer-1 by one 16-step chunk; projection
matmuls interleave into the scan stream to fill PE gaps.
"""
import sys

sys.path.insert(0, "/opt/trn_rl_repo")
sys.path.insert(0, "/opt/trn_rl_repo/concourse")

import numpy as np

import concourse.bass as bass
import concourse.bacc as bacc
import concourse.tile as tile
from concourse import mybir
from concourse.bass_utils import run_bass_kernel_spmd
from concourse.masks import make_identity

F32 = mybir.dt.float32
BF16 = mybir.dt.bfloat16
NPBF16 = mybir.dt.np(BF16)
SIG = mybir.ActivationFunctionType.Sigmoid

N_CORES = 8
B_TOT, T_FULL, F_IN, U = 128, 512, 128, 512
B = B_TOT // N_CORES          # 16 local batch
G3 = 3 * U                    # 1536 gate cols
KT = U // 128                 # 4 k-tiles
GT = G3 // 128                # 12 gate tiles
CH = 16                       # steps per chunk
BTC = CH * B                  # 256 bt-cols per chunk


def build_bass(T=T_FULL, with_bi1=False, with_br1=False, with_bi2=False,
               with_br2=False):
    nc = bacc.Bacc("TRN2", target_bir_lowering=False, debug=False,
                   enable_asserts=False, num_devices=N_CORES)
    NCH = T // CH

    xT_d = nc.dram_tensor("xTb", [F_IN, B * T], BF16, kind="ExternalInput").ap()
    W1_d = nc.dram_tensor("W1b", [F_IN, G3], BF16, kind="ExternalInput").ap()
    U1_d = nc.dram_tensor("U1b", [U, G3], BF16, kind="ExternalInput").ap()
    W2_d = nc.dram_tensor("W2b", [U, G3], BF16, kind="ExternalInput").ap()
    U2_d = nc.dram_tensor("U2b", [U, G3], BF16, kind="ExternalInput").ap()
    Wd_d = nc.dram_tensor("Wdb", [U, 1], BF16, kind="ExternalInput").ap()
    bi1_d = nc.dram_tensor("bi1b", [1, G3], BF16, kind="ExternalInput").ap()
    br1_d = nc.dram_tensor("br1b", [1, G3], BF16, kind="ExternalInput").ap()
    bi2_d = nc.dram_tensor("bi2b", [1, G3], BF16, kind="ExternalInput").ap()
    br2_d = nc.dram_tensor("br2b", [1, G3], BF16, kind="ExternalInput").ap()
    bd_d = nc.dram_tensor("bd", [1], F32, kind="ExternalInput").ap()
    out_d = nc.dram_tensor("out", [B, 1], F32, kind="ExternalOutput").ap()

    with tile.TileContext(nc) as tc:
        from contextlib import ExitStack
        with ExitStack() as ctx:
            const = ctx.enter_context(tc.tile_pool(name="const", bufs=1))
            gx1p = ctx.enter_context(tc.tile_pool(name="gx1", bufs=2))
            gx2p = ctx.enter_context(tc.tile_pool(name="gx2", bufs=2))
            gatep = ctx.enter_context(tc.tile_pool(name="gates", bufs=4))
            ps1p = ctx.enter_context(tc.tile_pool(name="ps1", bufs=2, space="PSUM"))
            ps2p = ctx.enter_context(tc.tile_pool(name="ps2", bufs=2, space="PSUM"))
            pspp = ctx.enter_context(tc.tile_pool(name="psp", bufs=2, space="PSUM"))

            # ---- constants / weights
            idf = const.tile([128, 128], F32)
            make_identity(nc, idf)
            I128 = const.tile([128, 128], BF16)
            nc.vector.tensor_copy(I128[:], idf[:])

            W1sb = const.tile([128, G3], BF16)
            nc.sync.dma_start(W1sb[:], W1_d[:])
            U1sb = const.tile([128, KT, G3], BF16)
            nc.sync.dma_start(U1sb[:], U1_d.rearrange("(k p) g -> p k g", p=128))
            W2sb = const.tile([128, KT, G3], BF16)
            nc.sync.dma_start(W2sb[:], W2_d.rearrange("(k p) g -> p k g", p=128))
            U2sb = const.tile([128, KT, G3], BF16)
            nc.sync.dma_start(U2sb[:], U2_d.rearrange("(k p) g -> p k g", p=128))
            Wdsb = const.tile([128, KT, 1], BF16)
            nc.sync.dma_start(Wdsb[:], Wd_d.rearrange("(k p) o -> p k o", p=128))
            bdsb = const.tile([1, 1], F32)
            nc.sync.dma_start(bdsb[:], bd_d[None, :])

            xsb = const.tile([128, B * T], BF16)
            nc.sync.dma_start(xsb[:], xT_d[:])

            ones16 = None
            ones256 = None
            if with_br1 or with_br2:
                o16f = const.tile([1, B], F32)
                nc.vector.memset(o16f, 1.0)
                ones16 = const.tile([1, B], BF16)
                nc.vector.tensor_copy(ones16[:], o16f[:])
            if with_bi1 or with_bi2:
                o256f = const.tile([1, BTC], F32)
                nc.vector.memset(o256f, 1.0)
                ones256 = const.tile([1, BTC], BF16)
                nc.vector.tensor_copy(ones256[:], o256f[:])

            def _row(d, flag, tag):
                if not flag:
                    return None
                t = const.tile([1, G3], BF16, tag=tag)
                nc.sync.dma_start(t[:], d[:])
                return t

            bi1r = _row(bi1_d, with_bi1, "bi1")
            br1r = _row(br1_d, with_br1, "br1")
            bi2r = _row(bi2_d, with_bi2, "bi2")
            br2r = _row(br2_d, with_br2, "br2")

            # ---- state
            h1T = const.tile([128, KT, T + 1, B], BF16)
            nc.vector.memset(h1T[:, :, 0, :], 0.0)
            h2T = const.tile([128, KT, 2, B], BF16)
            nc.vector.memset(h2T[:, :, 0, :], 0.0)

            # ---- helpers
            def proj_alloc(which):
                pool, tag = (gx1p, "g1") if which == 1 else (gx2p, "g2")
                return pool.tile([128, GT, BTC], BF16, tag=tag, name=tag)

            def proj_unit(g, c, which, gi):
                """Emit one gate-tile worth of projection for chunk c."""
                birow = bi1r if which == 1 else bi2r
                pp = pspp.tile([128, 512], F32, tag="pp")
                if which == 1:
                    nc.tensor.matmul(
                        pp[:, 0:BTC], W1sb[:, gi * 128:(gi + 1) * 128],
                        xsb[:, c * BTC:(c + 1) * BTC],
                        start=True, stop=(birow is None))
                else:
                    for k in range(KT):
                        nc.tensor.matmul(
                            pp[:, 0:BTC], W2sb[:, k, gi * 128:(gi + 1) * 128],
                            h1T[:, k, 1 + c * CH:1 + (c + 1) * CH, :],
                            start=(k == 0),
                            stop=(k == KT - 1 and birow is None))
                if birow is not None:
                    nc.tensor.matmul(
                        pp[:, 0:BTC], birow[:, gi * 128:(gi + 1) * 128],
                        ones256[:], start=False, stop=True)
                if gi % 2 == 0:
                    nc.scalar.copy(g[:, gi, :], pp[:, 0:BTC])
                else:
                    nc.vector.tensor_copy(g[:, gi, :], pp[:, 0:BTC])

            def scan_step(s, ps_pool, Usb, gxt, tl, h_prev, h_next, brrow):
                ps = ps_pool.tile([128, 512], F32, tag=f"ps{s}")
                Z = ps[:, 0:64].rearrange("p (j b) -> p j b", j=KT)
                R = ps[:, 64:128].rearrange("p (j b) -> p j b", j=KT)
                H = ps[:, 128:192].rearrange("p (j b) -> p j b", j=KT)
                gsl = gxt[:, :, tl * B:(tl + 1) * B]
                # one accumulation epoch per step for the whole bank:
                # only the very first matmul clears has_written.
                # injects (I128 stationary loaded once)
                for j in range(KT):
                    nc.tensor.matmul(Z[:, j, :], I128[:], gsl[:, j, :],
                                     start=(j == 0), stop=False)
                for j in range(KT):
                    nc.tensor.matmul(R[:, j, :], I128[:], gsl[:, 4 + j, :],
                                     start=False, stop=False)
                if brrow is not None:
                    for bank, base in ((Z, 0), (R, 4), (H, 8)):
                        for j in range(KT):
                            gi = base + j
                            nc.tensor.matmul(
                                bank[:, j, :],
                                brrow[:, gi * 128:(gi + 1) * 128],
                                ones16[:], start=False, stop=False)
                # recurrence: Z bank, then R, then H; stop on final matmul
                for bank, base in ((Z, 0), (R, 4), (H, 8)):
                    for j in range(KT):
                        gi = base + j
                        for k in range(KT):
                            last = (base == 8 and j == KT - 1 and k == KT - 1)
                            nc.tensor.matmul(
                                bank[:, j, :],
                                Usb[:, k, gi * 128:(gi + 1) * 128],
                                h_prev[:, k, :], start=False, stop=last)
                    if base == 0:
                        z_sb = gatep.tile([128, KT, B], BF16, tag=f"z{s}")
                        nc.scalar.activation(z_sb[:], Z, SIG)
                        w_sb = gatep.tile([128, KT, B], BF16, tag=f"w{s}")
                        nc.scalar.activation(w_sb[:], Z, SIG, scale=-1.0)
                        zh = gatep.tile([128, KT, B], BF16, tag=f"zh{s}")
                        nc.gpsimd.tensor_mul(zh[:], z_sb[:], h_prev[:])
                    elif base == 4:
                        r_sb = gatep.tile([128, KT, B], F32, tag=f"r{s}")
                        nc.scalar.activation(r_sb[:], R, SIG)
                # candidate: hh = relu(gx_h + r * rec_h)
                t1 = gatep.tile([128, KT, B], BF16, tag=f"t1{s}")
                nc.vector.tensor_mul(t1[:], r_sb[:], H)
                t2 = gatep.tile([128, KT, B], BF16, tag=f"t2{s}")
                nc.vector.tensor_add(t2[:], t1[:], gsl[:, 8:12, :])
                # t3 = (1-z) * relu(t2), fused on DVE; h_new = t3 + z*h
                t3 = gatep.tile([128, KT, B], BF16, tag=f"t3{s}")
                nc.vector.scalar_tensor_tensor(
                    t3[:], t2[:], 0.0, w_sb[:],
                    mybir.AluOpType.max, mybir.AluOpType.mult)
                nc.vector.tensor_add(h_next[:], t3[:], zh[:])

            # ---- main pipeline: scan2 trails scan1 by one chunk;
            # projections run as blocks at chunk boundaries.
            for c in range(NCH + 1):
                if c < NCH:
                    g1 = proj_alloc(1)
                    for gi in range(GT):
                        proj_unit(g1, c, 1, gi)
                if c >= 1:
                    g2 = proj_alloc(2)
                    for gi in range(GT):
                        proj_unit(g2, c - 1, 2, gi)
                for tl in range(CH):
                    if c < NCH:
                        tg = c * CH + tl
                        scan_step(1, ps1p, U1sb, g1, tl,
                                  h1T[:, :, tg, :], h1T[:, :, tg + 1, :], br1r)
                    if c >= 1:
                        tg = (c - 1) * CH + tl
                        scan_step(2, ps2p, U2sb, g2, tl,
                                  h2T[:, :, tg % 2, :],
                                  h2T[:, :, (tg + 1) % 2, :], br2r)

            # ---- head: out = h2_last @ Wd + bd
            hp = pspp.tile([1, B], F32, tag="head")
            h_fin = h2T[:, :, T % 2, :]
            for k in range(KT):
                nc.tensor.matmul(hp[:], Wdsb[:, k, :], h_fin[:, k, :],
                                 start=(k == 0), stop=(k == KT - 1))
            res = const.tile([1, B], F32)
            nc.scalar.activation(res[:], hp[:],
                                 mybir.ActivationFunctionType.Identity,
                                 bias=bdsb[:])
            nc.sync.dma_start(out_d.rearrange("b o -> o b"), res[:])

    nc.compile()
    return nc


def prep_core_inputs(inputs, c):
    """Map the full-problem inputs to core c's dram tensors."""
    x = np.asarray(inputs["x"], np.float32)[c * B:(c + 1) * B]
    xT = np.ascontiguousarray(x.transpose(2, 1, 0).reshape(F_IN, B * T_FULL))
    m = {
        "xTb": xT.astype(NPBF16),
        "W1b": np.asarray(inputs["W1"], np.float32).astype(NPBF16),
        "U1b": np.asarray(inputs["U1"], np.float32).astype(NPBF16),
        "W2b": np.asarray(inputs["W2"], np.float32).astype(NPBF16),
        "U2b": np.asarray(inputs["U2"], np.float32).astype(NPBF16),
        "Wdb": np.asarray(inputs["Wd"], np.float32).astype(NPBF16),
        "bi1b": np.asarray(inputs["bi1"], np.float32).reshape(1, G3).astype(NPBF16),
        "br1b": np.asarray(inputs["br1"], np.float32).reshape(1, G3).astype(NPBF16),
        "bi2b": np.asarray(inputs["bi2"], np.float32).reshape(1, G3).astype(NPBF16),
        "br2b": np.asarray(inputs["br2"], np.float32).reshape(1, G3).astype(NPBF16),
        "bd": np.asarray(inputs["bd"], np.float32).reshape(1),
    }
    return m


_CACHED = {}


def _get_nc(key, **kw):
    if key not in _CACHED:
        _CACHED[key] = build_bass(**kw)
    return _CACHED[key]


def kernel(x, W1, U1, bi1, br1, W2, U2, bi2, br2, Wd, bd):
    inputs = dict(x=x, W1=W1, U1=U1, bi1=bi1, br1=br1, W2=W2, U2=U2,
                  bi2=bi2, br2=br2, Wd=Wd, bd=bd)
    kw = dict(
        with_bi1=bool(np.any(bi1)), with_br1=bool(np.any(br1)),
        with_bi2=bool(np.any(bi2)), with_br2=bool(np.any(br2)),
    )
    nc = _get_nc(("v2", T_FULL) + tuple(sorted(kw.items())), T=T_FULL, **kw)
    in_maps = [prep_core_inputs(inputs, c) for c in range(N_CORES)]
    res = run_bass_kernel_spmd(nc, in_maps, core_ids=list(range(N_CORES)))
    out = np.concatenate([res.results[c]["out"] for c in range(N_CORES)], axis=0)
    return out.astype(np.float32)
